# revision 39
# baseline (speedup 1.0000x reference)
"""Trainium2 Bass kernel for nn_CentralizedCritic (pooling critic net).

Data-parallel over 8 NeuronCores: each core handles B_c=2048 batch rows.

Per-core math (matches the jax reference):
  robot_emb = setenc(robot[b], rw*)  -> [B,32]   (mean+max pool over 64)
  track_emb = setenc(track[b], tw*)  -> [B,32]   (mean+max pool over 128)
  c = [tier0, robot_emb, track_emb]  -> [B,108]
  y = mlp(c)                         -> [B]

On-chip mapping:
  - Activations kept transposed [feat, rows]; 2 batch-halves packed on the
    partition dim via block-diag weights (K=2*d_in, M=2*d_hidden=128).
  - x^T pre-packed on host into 4 row-groups at partition offsets {0,32,64,96}
    so L1 matmuls row-tile the PE array. All matmuls bf16 (accum fp32).
  - relu+bias fused into the PSUM->SBUF evacuation (split ACT/DVE).
  - The whole set-encoder loop is SOFTWARE-PIPELINED at half-iter (2 quads x
    512 col) granularity: emission order is stage-skewed
    (L1(r) | evac1(r) | L2(r-1) evac2(r-1) | L3(r-2) pool(r-2)) so each
    in-order engine FIFO only receives ready work; PSUM rings:
    ps1 1x2 banks, ps2 2x2 banks, e/pool-sum 2x1 bank.
  - mean-pool: PE-side accumulating matmuls over SUM_F n-blocks (+ small DVE
    residual reduce); max-pool: DVE reduce_max from PSUM.
  - track-branch combine+scatter emitted mid-stream (hides under robot
    rounds); head MLP emission is stage-skewed across its 4 column tiles.
  - e-bias (rb3/tb3) folded into the head-L1 bias on host.
"""

import sys

sys.path.insert(0, "/opt/trn_rl_repo")

import numpy as np
import ml_dtypes

import concourse.bass as bass  # noqa: F401  (bass must import before tile)
import concourse.mybir as mybir
import concourse.tile as tile
from concourse import bacc
from concourse.bass_utils import run_bass_kernel_spmd

F32R = mybir.dt.float32r
F32 = mybir.dt.float32
BF16 = mybir.dt.bfloat16
AF = mybir.ActivationFunctionType
ALU = mybir.AluOpType
AX = mybir.AxisListType

N_CORES = 8
B = 16384
B_C = B // N_CORES          # 2048 batch rows per core
HALF = B_C // 2             # 1024 (2-row packing pairs b and b+HALF)
NR, DR = 64, 6              # robot set size / feature dim
NT, DT = 128, 7             # track set size / feature dim
CT = B_C * NT // 2          # 131072 packed track cols per core
CR = B_C * NR // 2          # 65536 packed robot cols per core
QT = CT // 4                # 32768 cols per track row-group
QR = CR // 4                # 16384 cols per robot row-group
CHUNK = int(__import__("os").environ.get("CHUNK", "4096"))  # dma chunk cols
NTILE = 512                 # matmul free dim
SUM_F = 8                   # accumulation groups for PE-side pooled-sum

# const-block column layout in "wts" [128, 840] (f32r)
W1T_C, W1R_C, W2T_C, W2R_C = 0, 128, 256, 384
MW1_C, MW2_C, MW3_C, MW4_C = 512, 640, 768, 832
WTS_W = 840
# "bs" [128, 8] (f32) bias columns
BS_TB1, BS_RB1, BS_TB2, BS_RB2, BS_MB1, BS_MB2, BS_MB3, BS_MB4 = range(8)

_CACHE = {}

import os
PROBE_NO_REDUCE = os.environ.get("PROBE_NO_REDUCE") == "1"
PROBE_NO_EVAC = os.environ.get("PROBE_NO_EVAC") == "1"
PROBE_NO_MM3 = os.environ.get("PROBE_NO_MM3") == "1"
HBUF_BUFS = int(os.environ.get("HBUF_BUFS", "3"))
XC_BUFS = int(os.environ.get("XC_BUFS", "3"))
PS1_BUFS = int(os.environ.get("PS1_BUFS", "1"))
PS2_BUFS = int(os.environ.get("PS2_BUFS", "2"))
PS3_BUFS = int(os.environ.get("PS3_BUFS", "2"))
DVE_EVAC_MOD = int(os.environ.get("DVE_EVAC_MOD", "3"))
MMDT_NAME = os.environ.get("MMDT", "bf16")
EVAC_ASSIGN = os.environ.get("EVAC_ASSIGN", "fixed")  # rr | e1 | fixed
# pooled-sum engine: every SUM_PE_MOD-th tile-iter computes the e-sum with
# PE accumulating matmuls (+ small DVE residual) instead of a DVE
# reduce_sum over the full e tile. 0 disables (all DVE).
SUM_PE_MOD = int(os.environ.get("SUM_PE_MOD", "1"))
SUM_PE_SKIP = int(os.environ.get("SUM_PE_SKIP", "3"))
POOL_FIRST = os.environ.get("POOL_FIRST", "0") == "1"
PSS_TAG = os.environ.get("PSS_TAG", "0") == "1"
SUM_PE_INV = os.environ.get("SUM_PE_INV", "0") == "1"
HEAD_DVE = os.environ.get("HEAD_DVE", "1") == "1"  # alternate head evacs
MMDT = mybir.dt.bfloat16 if MMDT_NAME == "bf16" else mybir.dt.float32r
MMDT_NP = ml_dtypes.bfloat16 if MMDT_NAME == "bf16" else np.float32


def _build_bass():
    nc = bacc.Bacc("TRN2", target_bir_lowering=False, debug=False,
                   num_devices=N_CORES)
    xt_d = nc.dram_tensor("xt", [128, QT], MMDT, kind="ExternalInput")
    xr_d = nc.dram_tensor("xr", [128, QR], MMDT, kind="ExternalInput")
    t0_d = nc.dram_tensor("t0", [44, B_C], MMDT, kind="ExternalInput")
    wts_d = nc.dram_tensor("wts", [128, WTS_W], MMDT, kind="ExternalInput")
    wbf_d = nc.dram_tensor("wbf", [128, 128], BF16, kind="ExternalInput")
    bs_d = nc.dram_tensor("bs", [128, 8], F32, kind="ExternalInput")
    y_d = nc.dram_tensor("y", [1, B_C], F32, kind="ExternalOutput")

    with tile.TileContext(nc) as tc:
        with (
            tc.tile_pool(name="consts", bufs=1) as consts,
            tc.tile_pool(name="xchunks", bufs=XC_BUFS) as xchunks,
            tc.tile_pool(name="hbuf", bufs=HBUF_BUFS) as hbuf,
            tc.tile_pool(name="acc", bufs=1) as acc,
            tc.tile_pool(name="head", bufs=2) as head,
            # PSUM plan (8 banks): ps1 ring 2x[128,2,512] = 4 banks (lives 2
            # rounds: L1(r)..evac1 emitted r+1), ps2 ring 1x = 2 banks
            # (consumed same round), e/pss ring 2x[128,512] = 2 banks.
            # Head reuses the ps2 ring.
            tc.tile_pool(name="ps1p", bufs=PS1_BUFS, space="PSUM") as ps1p,
            tc.tile_pool(name="ps2p", bufs=PS2_BUFS, space="PSUM") as ps2p,
            tc.tile_pool(name="ps3p", bufs=PS3_BUFS, space="PSUM") as ps3p,
        ):
            wts = consts.tile([128, WTS_W], MMDT)
            wbf = consts.tile([128, 128], BF16)
            bs = consts.tile([128, 8], F32)
            cT = acc.tile([108, B_C], MMDT)

            # pooled accumulators: col = 256*J + 32*ch + 4*t + om (track)
            #                      col = 256*J + 64*ch + 8*t + om (robot)
            esum_t = acc.tile([128, 512], F32, tag="esum_t")
            emax_t = acc.tile([128, 512], F32, tag="emax_t")
            esum_r = acc.tile([128, 512], F32, tag="esum_r")
            emax_r = acc.tile([128, 512], F32, tag="emax_r")

            # -------- software-pipelined emission ------------------------
            # Each half-iter (2 row-group quads x NTILE cols) is split into
            # stages; the driver below emits them skewed so every engine's
            # in-order FIFO only ever sees ready work:
            #   round r: L1(r) evac1(r) L2(r-1) evac2(r-1) L3(r-2) pool(r-2)
            stages = []
            chunk_srcs = []      # chunk gid -> (dram tensor, col offset)
            xc_tiles = {}        # chunk gid -> sbuf tile
            tile_state = {}      # (gid, t) -> {"h1":..., "h2":...}

            def emit_prefetch(gid):
                if gid >= len(chunk_srcs) or gid in xc_tiles:
                    return
                x_d, c0 = chunk_srcs[gid]
                xc = xchunks.tile([128, CHUNK], MMDT, tag="xc")
                if gid == 0:
                    # first chunk: 4 pieces so round-0 L1 starts after the
                    # first quarter lands instead of the full 5us transfer
                    qq = CHUNK // 4
                    for pc in range(4):
                        nc.sync.dma_start(
                            out=xc[:, pc * qq:(pc + 1) * qq],
                            in_=x_d[:, c0 + pc * qq:c0 + (pc + 1) * qq])
                else:
                    nc.sync.dma_start(out=xc[:], in_=x_d[:, c0:c0 + CHUNK])
                xc_tiles[gid] = xc

            def make_half(k, gid, t, half, k2, w1_c, w2_c, w3_c,
                          bs1, bs2, nseg, esum, emax, base):
                nb = NTILE // nseg
                cs = slice(t * NTILE, (t + 1) * NTILE)
                st = {}

                def evac_op(pshalf, htile, bias_col, use_dve):
                    pv = pshalf.rearrange("p a b -> p (a b)")
                    hv = htile[:, 2 * half:2 * half + 2, :].rearrange(
                        "p a b -> p (a b)")
                    if use_dve:
                        nc.vector.tensor_scalar(
                            out=hv[:], in0=pv[:],
                            scalar1=bs[:, bias_col:bias_col + 1],
                            scalar2=0.0, op0=ALU.add, op1=ALU.max)
                    else:
                        nc.scalar.activation(
                            out=hv[:], in_=pv[:], func=AF.Relu,
                            bias=bs[:, bias_col:bias_col + 1], scale=1.0)

                def s_l1():
                    ps1 = ps1p.tile([128, 2, NTILE], F32, tag="ps1",
                                    name="ps1")
                    st["ps1"] = ps1
                    xc = xc_tiles[gid]
                    for j in range(2):
                        q = 2 * half + j
                        nc.tensor.matmul(
                            ps1[:, j, :],
                            wts[32 * q:32 * q + k2, w1_c:w1_c + 128],
                            xc[32 * q:32 * q + k2, cs],
                            start=True, stop=True,
                            tile_position=(32 * q, 0),
                        )

                def s_evac1():
                    ts = tile_state.setdefault((gid, t), {})
                    if "h1" not in ts:
                        ts["h1"] = hbuf.tile([128, 4, NTILE], MMDT,
                                             tag="h1", name="h1")
                    evac_op(st["ps1"], ts["h1"], bs1, use_dve=False)

                def s_l2():
                    ts = tile_state[(gid, t)]
                    ps2 = ps2p.tile([128, 2, NTILE], F32, tag="ps2",
                                    name="ps2")
                    st["ps2"] = ps2
                    for j in range(2):
                        q = 2 * half + j
                        nc.tensor.matmul(
                            ps2[:, j, :],
                            wts[:, w2_c:w2_c + 128],
                            ts["h1"][:, q, :],
                            start=True, stop=True,
                        )

                def s_evac2():
                    ts = tile_state[(gid, t)]
                    if "h2" not in ts:
                        ts["h2"] = hbuf.tile([128, 4, NTILE], BF16,
                                             tag="h2", name="h2")
                    evac_op(st["ps2"], ts["h2"], bs2,
                            use_dve=(EVAC_ASSIGN == "fixed"
                                     and half == 1))

                def s_l3():
                    ts = tile_state[(gid, t)]
                    h2 = ts["h2"]
                    ps3 = ps3p.tile([128, nb * nseg], F32, tag="ps3",
                                    name="ps3")
                    st["ps3"] = ps3
                    for q in (2 * half, 2 * half + 1):
                        blk = q % 2
                        nc.tensor.matmul(
                            ps3[64 * blk:64 * blk + 64, :],
                            wbf[:, w3_c:w3_c + 64],
                            h2[:, q, :],
                            start=True, stop=True,
                            tile_position=(0, 64 * blk),
                        )
                    pe_sum_on = (SUM_PE_MOD > 0 and
                                 ((k % SUM_PE_MOD != 0) if SUM_PE_INV
                                  else (k % SUM_PE_MOD == 0)))
                    if pe_sum_on:
                        sc = nseg // SUM_F
                        if PSS_TAG:
                            pss = ps3p.tile([128, nb * sc], F32,
                                            tag="pss", name="pss")
                        else:
                            pss = ps3p.tile([128, nb * nseg], F32,
                                            tag="ps3", name="pss")
                        st["pss"] = pss
                        for q in (2 * half, 2 * half + 1):
                            blk = q % 2
                            rv = h2[:, q, :].rearrange(
                                "p (b j s) -> p j b s", b=nb, j=SUM_F)
                            ov = pss[64 * blk:64 * blk + 64,
                                     0:nb * sc].rearrange(
                                "p (b s) -> p b s", b=nb)
                            for j in range(SUM_F):
                                nc.tensor.matmul(
                                    ov, wbf[:, w3_c:w3_c + 64], rv[:, j],
                                    start=(j == 0),
                                    stop=(j == SUM_F - 1),
                                    tile_position=(0, 64 * blk),
                                )

                def s_pool():
                    ps3 = st["ps3"]
                    p3r = ps3.rearrange("p (b c) -> p b c", b=nb)
                    sview = esum.rearrange("p (J r) -> p J r", J=2)[
                        :, half:half + 1, base:base + nb]
                    mview = emax.rearrange("p (J r) -> p J r", J=2)[
                        :, half:half + 1, base:base + nb]
                    if PROBE_NO_REDUCE:
                        return
                    if "pss" in st:
                        sc = nseg // SUM_F
                        nc.vector.reduce_sum(
                            out=sview,
                            in_=st["pss"][:, 0:nb * sc].rearrange(
                                "p (b s) -> p b s", b=nb),
                            axis=AX.X)
                    else:
                        nc.vector.reduce_sum(out=sview, in_=p3r[:],
                                             axis=AX.X)
                    nc.vector.reduce_max(out=mview, in_=p3r[:], axis=AX.X)

                return {"l1": s_l1, "evac1": s_evac1, "l2": s_l2,
                        "evac2": s_evac2, "l3": s_l3, "pool": s_pool}

            def branch(x_d, qcols, k2, w1_c, w2_c, w3_c, bs1, bs2, nseg,
                       esum, emax):
                nchunks = qcols // CHUNK
                tpc = CHUNK // NTILE
                gid0 = len(chunk_srcs)
                for ch in range(nchunks):
                    chunk_srcs.append((x_d, ch * CHUNK))
                for ch in range(nchunks):
                    for t in range(tpc):
                        for half in range(2):
                            base = (32 * ch + 4 * t) if nseg == NT \
                                else (64 * ch + 8 * t)
                            stages.append(make_half(
                                len(stages), gid0 + ch, t, half, k2,
                                w1_c, w2_c, w3_c, bs1, bs2, nseg,
                                esum, emax, base))

            branch(xt_d, QT, 2 * DT, W1T_C, W2T_C, 0, BS_TB1, BS_TB2, NT,
                   esum_t, emax_t)
            branch(xr_d, QR, 2 * DR, W1R_C, W2R_C, 64, BS_RB1, BS_RB2, NR,
                   esum_r, emax_r)

            # ---- per-branch tail: combine + scatter into cT --------------
            emb_t = acc.tile([128, 512], MMDT, tag="emb_t")
            emb_r = acc.tile([128, 512], MMDT, tag="emb_r")

            def branch_tail(esum, emax, emb, nseg, row0):
                """emb = esum/(2*nseg) + 0.5*emax, then scatter 32-part
                blocks into cT. Emitted as soon as the branch's last pool
                stage is out, so the track tail hides under robot rounds."""
                tmp = hbuf.tile([128, 512], F32, tag="tmp", name="tmp")
                nc.vector.tensor_scalar(out=tmp[:], in0=esum[:],
                                        scalar1=1.0 / (2.0 * nseg),
                                        scalar2=None, op0=ALU.mult)
                nc.vector.scalar_tensor_tensor(
                    out=emb[:], in0=emax[:], scalar=0.5, in1=tmp[:],
                    op0=ALU.mult, op1=ALU.add)
                # one DMA per (blk, h): [32, 2, 256] J-major view;
                # cT col = 1024*h + 512*J + 256*blk + c
                cv = cT[row0:row0 + 32, :].rearrange(
                    "p (h J blk c) -> p h J blk c", h=2, J=2, blk=2)
                for blk in range(2):
                    for h in range(2):
                        src = emb[64 * blk + 32 * h:64 * blk + 32 * h + 32,
                                  :].rearrange("p (J c) -> p J c", J=2)
                        nc.sync.dma_start(out=cv[:, h, :, blk, :], in_=src)

            HPC = 2 * (CHUNK // NTILE)   # half-iters per chunk = 16
            PF = int(os.environ.get("PF", "8"))  # prefetch lead (rounds)
            TRACK_END = 2 * (QT // NTILE)        # 128 track half-iters
            # x chunk 0 first on the DMA queue (its first piece gates round
            # 0), then the consts in first-use order
            emit_prefetch(0)
            nc.sync.dma_start(out=wts[:], in_=wts_d[:])
            nc.sync.dma_start(out=bs[:], in_=bs_d[:])
            nc.sync.dma_start(out=wbf[:], in_=wbf_d[:])
            nc.sync.dma_start(out=cT[0:44, :], in_=t0_d[:])
            nst = len(stages)
            SK = int(os.environ.get("SKEW_E1", "0"))  # extra evac1 skew
            for r in range(nst + 2 + SK):
                if (r + PF) % HPC == 0:
                    emit_prefetch((r + PF) // HPC)
                if r < nst:
                    stages[r]["l1"]()
                if 0 <= r - SK < nst:
                    stages[r - SK]["evac1"]()
                if POOL_FIRST:
                    if 0 <= r - 2 - SK < nst:
                        stages[r - 2 - SK]["l3"]()
                        stages[r - 2 - SK]["pool"]()
                    if 0 <= r - 1 - SK < nst:
                        stages[r - 1 - SK]["l2"]()
                        stages[r - 1 - SK]["evac2"]()
                else:
                    if 0 <= r - 1 - SK < nst:
                        stages[r - 1 - SK]["l2"]()
                        stages[r - 1 - SK]["evac2"]()
                    if 0 <= r - 2 - SK < nst:
                        stages[r - 2 - SK]["l3"]()
                        stages[r - 2 - SK]["pool"]()
                if r - 2 - SK == TRACK_END - 1:
                    branch_tail(esum_t, emax_t, emb_t, NT, 76)
            branch_tail(esum_r, emax_r, emb_r, NR, 44)

            # keep the PE warm through the scatter window: harmless matmuls
            # on resident weights into a scratch psum slot
            for wi in range(4):
                pw = ps2p.tile([128, NTILE], F32, tag="ps2", name="pw")
                nc.tensor.matmul(pw[:], wts[:, MW2_C:MW2_C + 128],
                                 wts[:, 0:NTILE], start=True, stop=True)

            # ---- head MLP 108 -> 128 -> 128 -> 64 -> 1, stage-skewed ----
            y_sb = acc.tile([1, B_C], F32, tag="y")

            def hevac(out, in_, bcol, nrow=128, dve=False):
                if dve and HEAD_DVE:
                    nc.vector.tensor_scalar(
                        out=out, in0=in_,
                        scalar1=bs[0:nrow, bcol:bcol + 1],
                        scalar2=0.0, op0=ALU.add, op1=ALU.max)
                else:
                    nc.scalar.activation(
                        out=out, in_=in_, func=AF.Relu,
                        bias=bs[0:nrow, bcol:bcol + 1], scale=1.0)

            NHT = B_C // NTILE
            hstate = [dict() for _ in range(NHT)]

            def h_l1(t):
                s = hstate[t]
                s["psA"] = ps2p.tile([128, NTILE], F32, tag="ps2",
                                     name="psA")
                nc.tensor.matmul(s["psA"][:], wts[0:108, MW1_C:MW1_C + 128],
                                 cT[:, t * NTILE:(t + 1) * NTILE],
                                 start=True, stop=True)

            def h_l2(t):
                s = hstate[t]
                s["hh1"] = head.tile([128, NTILE], MMDT, tag="hh1",
                                     name="hh1")
                hevac(s["hh1"][:], s["psA"][:], BS_MB1)
                s["psB"] = ps1p.tile([128, NTILE], F32, tag="ps1",
                                     name="psB")
                nc.tensor.matmul(s["psB"][:], wts[:, MW2_C:MW2_C + 128],
                                 s["hh1"][:], start=True, stop=True)

            def h_l3(t):
                s = hstate[t]
                s["hh2"] = head.tile([128, NTILE], MMDT, tag="hh2",
                                     name="hh2")
                hevac(s["hh2"][:], s["psB"][:], BS_MB2, dve=True)
                s["psC"] = ps3p.tile([64, NTILE], F32, tag="ps3",
                                     name="psC")
                nc.tensor.matmul(s["psC"][:], wts[:, MW3_C:MW3_C + 64],
                                 s["hh2"][:], start=True, stop=True)

            def h_l4(t):
                s = hstate[t]
                s["hh3"] = head.tile([64, NTILE], MMDT, tag="hh3",
                                     name="hh3")
                hevac(s["hh3"][:], s["psC"][:], BS_MB3, nrow=64)
                s["psD"] = ps3p.tile([1, NTILE], F32, tag="ps3",
                                     name="psD")
                nc.tensor.matmul(s["psD"][:], wts[0:64, MW4_C:MW4_C + 1],
                                 s["hh3"][:], start=True, stop=True)

            def h_fin(t):
                s = hstate[t]
                cs = slice(t * NTILE, (t + 1) * NTILE)
                nc.vector.tensor_scalar(out=y_sb[:, cs], in0=s["psD"][:],
                                        scalar1=bs[0:1, BS_MB4:BS_MB4 + 1],
                                        scalar2=None, op0=ALU.add)
                s.clear()

            hstages = [h_l1, h_l2, h_l3, h_l4, h_fin]
            for r in range(NHT + len(hstages) - 1):
                for si, fn in enumerate(hstages):
                    t = r - si
                    if 0 <= t < NHT:
                        fn(t)
            nc.sync.dma_start(out=y_d[:], in_=y_sb[:])

    nc.compile()
    return nc


def _pack_x(x, d, qcols):
    """x [rows, d] (rows = B_c*nseg, b-major) -> [128, qcols] with 4
    row-groups at partition offsets {0,32,64,96}; 2-row packing pairs
    row r with row r + rows/2."""
    rows = x.shape[0]
    half = rows // 2
    packed = np.concatenate([x[:half].T, x[half:].T], axis=0)  # [2d, half]
    out = np.zeros((128, qcols), dtype=MMDT_NP)
    for q in range(4):
        out[32 * q:32 * q + 2 * d] = packed[:, q * qcols:(q + 1) * qcols]
    return np.ascontiguousarray(out)


def _blockdiag2(w):
    """w [d, m] -> [2d, 2m] block-diagonal."""
    d, m = w.shape
    out = np.zeros((2 * d, 2 * m), dtype=np.float32)
    out[:d, :m] = w
    out[d:, m:] = w
    return out


def _build_consts(i):
    np32 = lambda a: np.asarray(a, dtype=np.float32)
    wts = np.zeros((128, WTS_W), dtype=np.float32)
    # L1 lhsT blocks replicated at the 4 row-group offsets
    bd1t = _blockdiag2(np32(i["tw1"]))   # [14, 128]
    bd1r = _blockdiag2(np32(i["rw1"]))   # [12, 128]
    for q in range(4):
        wts[32 * q:32 * q + 14, W1T_C:W1T_C + 128] = bd1t
        wts[32 * q:32 * q + 12, W1R_C:W1R_C + 128] = bd1r
    wts[:, W2T_C:W2T_C + 128] = _blockdiag2(np32(i["tw2"]))
    wts[:, W2R_C:W2R_C + 128] = _blockdiag2(np32(i["rw2"]))
    wts[0:108, MW1_C:MW1_C + 128] = np32(i["mw1"])
    wts[:, MW2_C:MW2_C + 128] = np32(i["mw2"])
    wts[:, MW3_C:MW3_C + 64] = np32(i["mw3"])
    wts[0:64, MW4_C:MW4_C + 1] = np32(i["mw4"])
    wts = wts.astype(MMDT_NP)

    wbf = np.zeros((128, 128), dtype=np.float32)
    wbf[:, 0:64] = _blockdiag2(np32(i["tw3"]))
    wbf[:, 64:128] = _blockdiag2(np32(i["rw3"]))
    wbf = wbf.astype(ml_dtypes.bfloat16)

    bs = np.zeros((128, 8), dtype=np.float32)
    bs[:, BS_TB1] = np.concatenate([np32(i["tb1"]), np32(i["tb1"])])
    bs[:, BS_RB1] = np.concatenate([np32(i["rb1"]), np32(i["rb1"])])
    bs[:, BS_TB2] = np.concatenate([np32(i["tb2"]), np32(i["tb2"])])
    bs[:, BS_RB2] = np.concatenate([np32(i["rb2"]), np32(i["rb2"])])
    # fold pooled e-bias into head L1 bias: c@mw1 picks up b3@mw1 rows
    mb1p = (np32(i["mb1"])
            + np32(i["rb3"]) @ np32(i["mw1"])[44:76]
            + np32(i["tb3"]) @ np32(i["mw1"])[76:108])
    bs[:, BS_MB1] = mb1p
    bs[:, BS_MB2] = np32(i["mb2"])
    bs[0:64, BS_MB3] = np32(i["mb3"])
    bs[0:1, BS_MB4] = np32(i["mb4"])
    return wts, wbf, bs


def kernel(**inputs) -> np.ndarray:
    if "nc" not in _CACHE:
        _CACHE["nc"] = _build_bass()
    nc = _CACHE["nc"]

    wts, wbf, bs = _build_consts(inputs)
    t0 = np.asarray(inputs["tier0_features"], dtype=np.float32)
    rb = np.asarray(inputs["robot_features"], dtype=np.float32)
    tk = np.asarray(inputs["track_features"], dtype=np.float32)

    in_maps = []
    for c in range(N_CORES):
        s = slice(c * B_C, (c + 1) * B_C)
        in_maps.append({
            "xt": _pack_x(tk[s].reshape(B_C * NT, DT), DT, QT),
            "xr": _pack_x(rb[s].reshape(B_C * NR, DR), DR, QR),
            "t0": np.ascontiguousarray(t0[s].T).astype(MMDT_NP),
            "wts": wts, "wbf": wbf, "bs": bs,
        })

    res = run_bass_kernel_spmd(nc, in_maps, core_ids=list(range(N_CORES)))
    out = np.concatenate([r["y"][0] for r in res.results])
    return out.astype(np.float32)


if __name__ == "__main__":
    rng = np.random.default_rng(0)
    fake = {
        "tier0_features": rng.standard_normal((B, 44), dtype=np.float32),
        "robot_features": rng.standard_normal((B, NR, DR), dtype=np.float32),
        "track_features": rng.standard_normal((B, NT, DT), dtype=np.float32),
    }
    for n, sh in (("rw1", (6, 64)), ("rw2", (64, 64)), ("rw3", (64, 32)),
                  ("tw1", (7, 64)), ("tw2", (64, 64)), ("tw3", (64, 32)),
                  ("mw1", (108, 128)), ("mw2", (128, 128)),
                  ("mw3", (128, 64)), ("mw4", (64, 1))):
        fake[n] = rng.standard_normal(sh, dtype=np.float32) * 0.2
    for n, sh in (("rb1", 64), ("rb2", 64), ("rb3", 32),
                  ("tb1", 64), ("tb2", 64), ("tb3", 32),
                  ("mb1", 128), ("mb2", 128), ("mb3", 64), ("mb4", 1)):
        fake[n] = rng.standard_normal((sh,), dtype=np.float32) * 0.1
    y = kernel(**fake)
    print("kernel out:", y.shape, y[:4])



# revision 41
# speedup vs baseline: 1.0368x; 1.0368x over previous
"""Trainium2 Bass kernel for nn_CentralizedCritic (pooling critic net).

Data-parallel over 8 NeuronCores: each core handles B_c=2048 batch rows.

Per-core math (matches the jax reference):
  robot_emb = setenc(robot[b], rw*)  -> [B,32]   (mean+max pool over 64)
  track_emb = setenc(track[b], tw*)  -> [B,32]   (mean+max pool over 128)
  c = [tier0, robot_emb, track_emb]  -> [B,108]
  y = mlp(c)                         -> [B]

On-chip mapping:
  - Activations kept transposed [feat, rows]; 2 batch-halves packed on the
    partition dim via block-diag weights (K=2*d_in, M=2*d_hidden=128).
  - x^T pre-packed on host into 4 row-groups at partition offsets {0,32,64,96}
    so L1 matmuls row-tile the PE array. All matmuls bf16 (accum fp32).
  - relu+bias fused into the PSUM->SBUF evacuation (split ACT/DVE).
  - The whole set-encoder loop is SOFTWARE-PIPELINED at half-iter (2 quads x
    512 col) granularity: emission order is stage-skewed
    (L1(r) | evac1(r) | L2(r-1) evac2(r-1) | L3(r-2) pool(r-2)) so each
    in-order engine FIFO only receives ready work; PSUM rings:
    ps1 1x2 banks, ps2 2x2 banks, e/pool-sum 2x1 bank.
  - mean-pool: PE-side accumulating matmuls over SUM_F n-blocks (+ small DVE
    residual reduce); max-pool: DVE reduce_max from PSUM.
  - track-branch combine+scatter emitted mid-stream (hides under robot
    rounds); head MLP emission is stage-skewed across its 4 column tiles.
  - e-bias (rb3/tb3) folded into the head-L1 bias on host.
"""

import sys

sys.path.insert(0, "/opt/trn_rl_repo")

import numpy as np
import ml_dtypes

import concourse.bass as bass  # noqa: F401  (bass must import before tile)
import concourse.mybir as mybir
import concourse.tile as tile
from concourse import bacc
from concourse.bass_utils import run_bass_kernel_spmd

F32R = mybir.dt.float32r
F32 = mybir.dt.float32
BF16 = mybir.dt.bfloat16
E4 = mybir.dt.float8e4
E4NP = ml_dtypes.float8_e4m3
PM = mybir.MatmulPerfMode
AF = mybir.ActivationFunctionType
ALU = mybir.AluOpType
AX = mybir.AxisListType

N_CORES = 8
B = 16384
B_C = B // N_CORES          # 2048 batch rows per core
HALF = B_C // 2             # 1024 (2-row packing pairs b and b+HALF)
NR, DR = 64, 6              # robot set size / feature dim
NT, DT = 128, 7             # track set size / feature dim
CT = B_C * NT // 2          # 131072 packed track cols per core
CR = B_C * NR // 2          # 65536 packed robot cols per core
QT = CT // 4                # 32768 cols per track row-group
QR = CR // 4                # 16384 cols per robot row-group
CHUNK = int(__import__("os").environ.get("CHUNK", "4096"))  # dma chunk cols
NTILE = 512                 # matmul free dim
SUM_F = 8                   # accumulation groups for PE-side pooled-sum

# const-block column layout in "wts" [128, 840] (f32r)
W1T_C, W1R_C, W2T_C, W2R_C = 0, 128, 256, 384
MW1_C, MW2_C, MW3_C, MW4_C = 512, 640, 768, 832
WTS_W = 840
# "bs" [128, 8] (f32) bias columns
BS_TB1, BS_RB1, BS_TB2, BS_RB2, BS_MB1, BS_MB2, BS_MB3, BS_MB4 = range(8)

_CACHE = {}

import os
PROBE_NO_REDUCE = os.environ.get("PROBE_NO_REDUCE") == "1"
PROBE_NO_EVAC = os.environ.get("PROBE_NO_EVAC") == "1"
PROBE_NO_MM3 = os.environ.get("PROBE_NO_MM3") == "1"
HBUF_BUFS = int(os.environ.get("HBUF_BUFS", "3"))
XC_BUFS = int(os.environ.get("XC_BUFS", "3"))
PS1_BUFS = int(os.environ.get("PS1_BUFS", "1"))
PS2_BUFS = int(os.environ.get("PS2_BUFS", "2"))
PS3_BUFS = int(os.environ.get("PS3_BUFS", "2"))
DVE_EVAC_MOD = int(os.environ.get("DVE_EVAC_MOD", "3"))
MMDT_NAME = os.environ.get("MMDT", "bf16")
EVAC_ASSIGN = os.environ.get("EVAC_ASSIGN", "fixed")  # rr | e1 | fixed
# pooled-sum engine: every SUM_PE_MOD-th tile-iter computes the e-sum with
# PE accumulating matmuls (+ small DVE residual) instead of a DVE
# reduce_sum over the full e tile. 0 disables (all DVE).
SUM_PE_MOD = int(os.environ.get("SUM_PE_MOD", "1"))
SUM_PE_SKIP = int(os.environ.get("SUM_PE_SKIP", "3"))
POOL_FIRST = os.environ.get("POOL_FIRST", "0") == "1"
PSS_TAG = os.environ.get("PSS_TAG", "0") == "1"
SUM_PE_INV = os.environ.get("SUM_PE_INV", "0") == "1"
HEAD_DVE = os.environ.get("HEAD_DVE", "1") == "1"  # alternate head evacs
MMDT = mybir.dt.bfloat16 if MMDT_NAME == "bf16" else mybir.dt.float32r
MMDT_NP = ml_dtypes.bfloat16 if MMDT_NAME == "bf16" else np.float32


def _build_bass():
    nc = bacc.Bacc("TRN2", target_bir_lowering=False, debug=False,
                   num_devices=N_CORES)
    xt_d = nc.dram_tensor("xt", [128, 2 * QT], E4, kind="ExternalInput")
    xr_d = nc.dram_tensor("xr", [128, 2 * QR], E4, kind="ExternalInput")
    w18_d = nc.dram_tensor("w18", [128, 512], E4, kind="ExternalInput")
    t0_d = nc.dram_tensor("t0", [44, B_C], MMDT, kind="ExternalInput")
    wts_d = nc.dram_tensor("wts", [128, WTS_W], MMDT, kind="ExternalInput")
    wbf_d = nc.dram_tensor("wbf", [128, 128], BF16, kind="ExternalInput")
    bs_d = nc.dram_tensor("bs", [128, 8], F32, kind="ExternalInput")
    y_d = nc.dram_tensor("y", [1, B_C], F32, kind="ExternalOutput")

    with tile.TileContext(nc) as tc:
        with (
            tc.tile_pool(name="consts", bufs=1) as consts,
            tc.tile_pool(name="xchunks", bufs=XC_BUFS) as xchunks,
            tc.tile_pool(name="hbuf", bufs=HBUF_BUFS) as hbuf,
            tc.tile_pool(name="acc", bufs=1) as acc,
            tc.tile_pool(name="head", bufs=2) as head,
            # PSUM plan (8 banks): ps1 ring 2x[128,2,512] = 4 banks (lives 2
            # rounds: L1(r)..evac1 emitted r+1), ps2 ring 1x = 2 banks
            # (consumed same round), e/pss ring 2x[128,512] = 2 banks.
            # Head reuses the ps2 ring.
            tc.tile_pool(name="ps1p", bufs=PS1_BUFS, space="PSUM") as ps1p,
            tc.tile_pool(name="ps2p", bufs=PS2_BUFS, space="PSUM") as ps2p,
            tc.tile_pool(name="ps3p", bufs=PS3_BUFS, space="PSUM") as ps3p,
        ):
            wts = consts.tile([128, WTS_W], MMDT)
            w18 = consts.tile([128, 512], E4)
            wbf = consts.tile([128, 128], BF16)
            bs = consts.tile([128, 8], F32)
            cT = acc.tile([108, B_C], MMDT)

            # pooled accumulators: col = 256*J + 32*ch + 4*t + om (track)
            #                      col = 256*J + 64*ch + 8*t + om (robot)
            esum_t = acc.tile([128, 512], F32, tag="esum_t")
            emax_t = acc.tile([128, 512], F32, tag="emax_t")
            esum_r = acc.tile([128, 512], F32, tag="esum_r")
            emax_r = acc.tile([128, 512], F32, tag="emax_r")

            # -------- software-pipelined emission ------------------------
            # Each half-iter (2 row-group quads x NTILE cols) is split into
            # stages; the driver below emits them skewed so every engine's
            # in-order FIFO only ever sees ready work:
            #   round r: L1(r) evac1(r) L2(r-1) evac2(r-1) L3(r-2) pool(r-2)
            stages = []
            chunk_srcs = []      # chunk gid -> (dram tensor, col offset)
            xc_tiles = {}        # chunk gid -> sbuf tile
            tile_state = {}      # (gid, t) -> {"h1":..., "h2":...}

            def emit_prefetch(gid):
                if gid >= len(chunk_srcs) or gid in xc_tiles:
                    return
                x_d, c0 = chunk_srcs[gid]
                xc = xchunks.tile([128, 2 * CHUNK], E4, tag="xc")
                if gid == 0:
                    # first chunk: 4 pieces so round-0 L1 starts after the
                    # first quarter lands instead of the full transfer
                    qq = CHUNK // 2
                    for pc in range(4):
                        nc.sync.dma_start(
                            out=xc[:, pc * qq:(pc + 1) * qq],
                            in_=x_d[:, 2 * c0 + pc * qq:
                                     2 * c0 + (pc + 1) * qq])
                else:
                    nc.sync.dma_start(out=xc[:],
                                      in_=x_d[:, 2 * c0:2 * c0 + 2 * CHUNK])
                xc_tiles[gid] = xc

            def make_half(k, gid, t, half, k2, w1_c, w2_c, w3_c,
                          bs1, bs2, nseg, esum, emax, base):
                nb = NTILE // nseg
                cs = slice(t * NTILE, (t + 1) * NTILE)
                st = {}

                def evac_op(pshalf, htile, bias_col, use_dve):
                    pv = pshalf.rearrange("p a b -> p (a b)")
                    hv = htile[:, 2 * half:2 * half + 2, :].rearrange(
                        "p a b -> p (a b)")
                    if use_dve:
                        nc.vector.tensor_scalar(
                            out=hv[:], in0=pv[:],
                            scalar1=bs[:, bias_col:bias_col + 1],
                            scalar2=0.0, op0=ALU.add, op1=ALU.max)
                    else:
                        nc.scalar.activation(
                            out=hv[:], in_=pv[:], func=AF.Relu,
                            bias=bs[:, bias_col:bias_col + 1], scale=1.0)

                def s_l1():
                    ps1 = ps1p.tile([128, 2, NTILE], F32, tag="ps1",
                                    name="ps1")
                    st["ps1"] = ps1
                    xc = xc_tiles[gid]
                    d4 = 2 * k2
                    w8c = 0 if w1_c == W1T_C else 256
                    for j in range(2):
                        q = 2 * half + j
                        rhs = xc[32 * q:32 * q + d4,
                                 2 * cs.start:2 * cs.stop].rearrange(
                            "p (n two) -> p two n", two=2)
                        lhsT = w18[32 * q:32 * q + d4,
                                   w8c:w8c + 256].rearrange(
                            "p (two m) -> p two m", two=2)
                        nc.tensor.matmul(
                            ps1[:, j, :], lhsT, rhs,
                            start=True, stop=True,
                            perf_mode=PM.DoubleRow,
                            tile_position=(32 * q, 0),
                        )

                def s_evac1():
                    ts = tile_state.setdefault((gid, t), {})
                    if "h1" not in ts:
                        ts["h1"] = hbuf.tile([128, 4, NTILE], MMDT,
                                             tag="h1", name="h1")
                    evac_op(st["ps1"], ts["h1"], bs1, use_dve=False)

                def s_l2():
                    ts = tile_state[(gid, t)]
                    ps2 = ps2p.tile([128, 2, NTILE], F32, tag="ps2",
                                    name="ps2")
                    st["ps2"] = ps2
                    for j in range(2):
                        q = 2 * half + j
                        nc.tensor.matmul(
                            ps2[:, j, :],
                            wts[:, w2_c:w2_c + 128],
                            ts["h1"][:, q, :],
                            start=True, stop=True,
                        )

                def s_evac2():
                    ts = tile_state[(gid, t)]
                    if "h2" not in ts:
                        ts["h2"] = hbuf.tile([128, 4, NTILE], BF16,
                                             tag="h2", name="h2")
                    evac_op(st["ps2"], ts["h2"], bs2,
                            use_dve=(EVAC_ASSIGN == "fixed"
                                     and half == 1))

                def s_l3():
                    ts = tile_state[(gid, t)]
                    h2 = ts["h2"]
                    ps3 = ps3p.tile([128, nb * nseg], F32, tag="ps3",
                                    name="ps3")
                    st["ps3"] = ps3
                    for q in (2 * half, 2 * half + 1):
                        blk = q % 2
                        nc.tensor.matmul(
                            ps3[64 * blk:64 * blk + 64, :],
                            wbf[:, w3_c:w3_c + 64],
                            h2[:, q, :],
                            start=True, stop=True,
                            tile_position=(0, 64 * blk),
                        )
                    pe_sum_on = (SUM_PE_MOD > 0 and
                                 ((k % SUM_PE_MOD != 0) if SUM_PE_INV
                                  else (k % SUM_PE_MOD == 0)))
                    if pe_sum_on:
                        sc = nseg // SUM_F
                        if PSS_TAG:
                            pss = ps3p.tile([128, nb * sc], F32,
                                            tag="pss", name="pss")
                        else:
                            pss = ps3p.tile([128, nb * nseg], F32,
                                            tag="ps3", name="pss")
                        st["pss"] = pss
                        for q in (2 * half, 2 * half + 1):
                            blk = q % 2
                            rv = h2[:, q, :].rearrange(
                                "p (b j s) -> p j b s", b=nb, j=SUM_F)
                            ov = pss[64 * blk:64 * blk + 64,
                                     0:nb * sc].rearrange(
                                "p (b s) -> p b s", b=nb)
                            for j in range(SUM_F):
                                nc.tensor.matmul(
                                    ov, wbf[:, w3_c:w3_c + 64], rv[:, j],
                                    start=(j == 0),
                                    stop=(j == SUM_F - 1),
                                    tile_position=(0, 64 * blk),
                                )

                def s_pool():
                    ps3 = st["ps3"]
                    p3r = ps3.rearrange("p (b c) -> p b c", b=nb)
                    sview = esum.rearrange("p (J r) -> p J r", J=2)[
                        :, half:half + 1, base:base + nb]
                    mview = emax.rearrange("p (J r) -> p J r", J=2)[
                        :, half:half + 1, base:base + nb]
                    if PROBE_NO_REDUCE:
                        return
                    if "pss" in st:
                        sc = nseg // SUM_F
                        nc.vector.reduce_sum(
                            out=sview,
                            in_=st["pss"][:, 0:nb * sc].rearrange(
                                "p (b s) -> p b s", b=nb),
                            axis=AX.X)
                    else:
                        nc.vector.reduce_sum(out=sview, in_=p3r[:],
                                             axis=AX.X)
                    nc.vector.reduce_max(out=mview, in_=p3r[:], axis=AX.X)

                return {"l1": s_l1, "evac1": s_evac1, "l2": s_l2,
                        "evac2": s_evac2, "l3": s_l3, "pool": s_pool}

            def branch(x_d, qcols, k2, w1_c, w2_c, w3_c, bs1, bs2, nseg,
                       esum, emax):
                nchunks = qcols // CHUNK
                tpc = CHUNK // NTILE
                gid0 = len(chunk_srcs)
                for ch in range(nchunks):
                    chunk_srcs.append((x_d, ch * CHUNK))
                for ch in range(nchunks):
                    for t in range(tpc):
                        for half in range(2):
                            base = (32 * ch + 4 * t) if nseg == NT \
                                else (64 * ch + 8 * t)
                            stages.append(make_half(
                                len(stages), gid0 + ch, t, half, k2,
                                w1_c, w2_c, w3_c, bs1, bs2, nseg,
                                esum, emax, base))

            branch(xt_d, QT, 2 * DT, W1T_C, W2T_C, 0, BS_TB1, BS_TB2, NT,
                   esum_t, emax_t)
            branch(xr_d, QR, 2 * DR, W1R_C, W2R_C, 64, BS_RB1, BS_RB2, NR,
                   esum_r, emax_r)

            # ---- per-branch tail: combine + scatter into cT --------------
            emb_t = acc.tile([128, 512], MMDT, tag="emb_t")
            emb_r = acc.tile([128, 512], MMDT, tag="emb_r")

            def branch_tail(esum, emax, emb, nseg, row0):
                """emb = esum/(2*nseg) + 0.5*emax, then scatter 32-part
                blocks into cT. Emitted as soon as the branch's last pool
                stage is out, so the track tail hides under robot rounds."""
                tmp = hbuf.tile([128, 512], F32, tag="tmp", name="tmp")
                nc.vector.tensor_scalar(out=tmp[:], in0=esum[:],
                                        scalar1=1.0 / (2.0 * nseg),
                                        scalar2=None, op0=ALU.mult)
                nc.vector.scalar_tensor_tensor(
                    out=emb[:], in0=emax[:], scalar=0.5, in1=tmp[:],
                    op0=ALU.mult, op1=ALU.add)
                # one DMA per (blk, h): [32, 2, 256] J-major view;
                # cT col = 1024*h + 512*J + 256*blk + c
                cv = cT[row0:row0 + 32, :].rearrange(
                    "p (h J blk c) -> p h J blk c", h=2, J=2, blk=2)
                for blk in range(2):
                    for h in range(2):
                        src = emb[64 * blk + 32 * h:64 * blk + 32 * h + 32,
                                  :].rearrange("p (J c) -> p J c", J=2)
                        nc.sync.dma_start(out=cv[:, h, :, blk, :], in_=src)

            HPC = 2 * (CHUNK // NTILE)   # half-iters per chunk = 16
            PF = int(os.environ.get("PF", "8"))  # prefetch lead (rounds)
            TRACK_END = 2 * (QT // NTILE)        # 128 track half-iters
            # x chunk 0 first on the DMA queue (its first piece gates round
            # 0), then the consts in first-use order
            emit_prefetch(0)
            nc.sync.dma_start(out=wts[:], in_=wts_d[:])
            nc.sync.dma_start(out=w18[:], in_=w18_d[:])
            nc.sync.dma_start(out=bs[:], in_=bs_d[:])
            nc.sync.dma_start(out=wbf[:], in_=wbf_d[:])
            nc.sync.dma_start(out=cT[0:44, :], in_=t0_d[:])
            nst = len(stages)
            SK = int(os.environ.get("SKEW_E1", "0"))  # extra evac1 skew
            for r in range(nst + 2 + SK):
                if (r + PF) % HPC == 0:
                    emit_prefetch((r + PF) // HPC)
                if r < nst:
                    stages[r]["l1"]()
                if 0 <= r - SK < nst:
                    stages[r - SK]["evac1"]()
                if POOL_FIRST:
                    if 0 <= r - 2 - SK < nst:
                        stages[r - 2 - SK]["l3"]()
                        stages[r - 2 - SK]["pool"]()
                    if 0 <= r - 1 - SK < nst:
                        stages[r - 1 - SK]["l2"]()
                        stages[r - 1 - SK]["evac2"]()
                else:
                    if 0 <= r - 1 - SK < nst:
                        stages[r - 1 - SK]["l2"]()
                        stages[r - 1 - SK]["evac2"]()
                    if 0 <= r - 2 - SK < nst:
                        stages[r - 2 - SK]["l3"]()
                        stages[r - 2 - SK]["pool"]()
                if r - 2 - SK == TRACK_END - 1:
                    branch_tail(esum_t, emax_t, emb_t, NT, 76)
            branch_tail(esum_r, emax_r, emb_r, NR, 44)

            # keep the PE warm through the scatter window: harmless matmuls
            # on resident weights into a scratch psum slot
            for wi in range(4):
                pw = ps2p.tile([128, NTILE], F32, tag="ps2", name="pw")
                nc.tensor.matmul(pw[:], wts[:, MW2_C:MW2_C + 128],
                                 wts[:, 0:NTILE], start=True, stop=True)

            # ---- head MLP 108 -> 128 -> 128 -> 64 -> 1, stage-skewed ----
            y_sb = acc.tile([1, B_C], F32, tag="y")

            def hevac(out, in_, bcol, nrow=128, dve=False):
                if dve and HEAD_DVE:
                    nc.vector.tensor_scalar(
                        out=out, in0=in_,
                        scalar1=bs[0:nrow, bcol:bcol + 1],
                        scalar2=0.0, op0=ALU.add, op1=ALU.max)
                else:
                    nc.scalar.activation(
                        out=out, in_=in_, func=AF.Relu,
                        bias=bs[0:nrow, bcol:bcol + 1], scale=1.0)

            NHT = B_C // NTILE
            hstate = [dict() for _ in range(NHT)]

            def h_l1(t):
                s = hstate[t]
                s["psA"] = ps2p.tile([128, NTILE], F32, tag="ps2",
                                     name="psA")
                nc.tensor.matmul(s["psA"][:], wts[0:108, MW1_C:MW1_C + 128],
                                 cT[:, t * NTILE:(t + 1) * NTILE],
                                 start=True, stop=True)

            def h_l2(t):
                s = hstate[t]
                s["hh1"] = head.tile([128, NTILE], MMDT, tag="hh1",
                                     name="hh1")
                hevac(s["hh1"][:], s["psA"][:], BS_MB1)
                s["psB"] = ps1p.tile([128, NTILE], F32, tag="ps1",
                                     name="psB")
                nc.tensor.matmul(s["psB"][:], wts[:, MW2_C:MW2_C + 128],
                                 s["hh1"][:], start=True, stop=True)

            def h_l3(t):
                s = hstate[t]
                s["hh2"] = head.tile([128, NTILE], MMDT, tag="hh2",
                                     name="hh2")
                hevac(s["hh2"][:], s["psB"][:], BS_MB2, dve=True)
                s["psC"] = ps3p.tile([64, NTILE], F32, tag="ps3",
                                     name="psC")
                nc.tensor.matmul(s["psC"][:], wts[:, MW3_C:MW3_C + 64],
                                 s["hh2"][:], start=True, stop=True)

            def h_l4(t):
                s = hstate[t]
                s["hh3"] = head.tile([64, NTILE], MMDT, tag="hh3",
                                     name="hh3")
                hevac(s["hh3"][:], s["psC"][:], BS_MB3, nrow=64)
                s["psD"] = ps3p.tile([1, NTILE], F32, tag="ps3",
                                     name="psD")
                nc.tensor.matmul(s["psD"][:], wts[0:64, MW4_C:MW4_C + 1],
                                 s["hh3"][:], start=True, stop=True)

            def h_fin(t):
                s = hstate[t]
                cs = slice(t * NTILE, (t + 1) * NTILE)
                nc.vector.tensor_scalar(out=y_sb[:, cs], in0=s["psD"][:],
                                        scalar1=bs[0:1, BS_MB4:BS_MB4 + 1],
                                        scalar2=None, op0=ALU.add)
                s.clear()

            hstages = [h_l1, h_l2, h_l3, h_l4, h_fin]
            for r in range(NHT + len(hstages) - 1):
                for si, fn in enumerate(hstages):
                    t = r - si
                    if 0 <= t < NHT:
                        fn(t)
            nc.sync.dma_start(out=y_d[:], in_=y_sb[:])

    nc.compile()
    return nc


def _hilo(a):
    a = np.asarray(a, np.float32)
    hi = a.astype(E4NP).astype(np.float32)
    lo = (a - hi).astype(E4NP).astype(np.float32)
    return hi, lo


def _pack_x(x, d, qcols):
    """x [rows, d] (rows = B_c*nseg, b-major) -> [128, 2*qcols] fp8.

    Pair (r, r+rows/2) goes to DoubleRow k-tiles t0/t1 (interleaved in the
    free dim); rows per quadrant are [x_hi(d); x_lo(d); x_hi; x_lo] so the
    [W_hi;W_hi;W_lo;W_lo] lhsT computes (W_hi+W_lo)(x_hi+x_lo) ~ exactly."""
    rows = x.shape[0]
    half = rows // 2
    ha, la = _hilo(x[:half])
    hb, lb = _hilo(x[half:])
    sa = np.concatenate([ha, la, ha, la], axis=1)      # [half, 4d]
    sb = np.concatenate([hb, lb, hb, lb], axis=1)
    inter = np.stack([sa, sb], axis=2)                 # [half, 4d, 2]
    out = np.zeros((128, 2 * qcols), dtype=E4NP)
    for q in range(4):
        blk = inter[q * qcols:(q + 1) * qcols].transpose(1, 0, 2)
        out[32 * q:32 * q + 4 * d] = blk.reshape(4 * d, 2 * qcols)
    return np.ascontiguousarray(out)


def _l1_lhsT(w1):
    """w1 [d, 64] -> [4d, 256] fp8 rows [Whi;Whi;Wlo;Wlo]; free col =
    two*128 + m: t0 -> out cols 0:64, t1 -> 64:128."""
    d = w1.shape[0]
    hi, lo = _hilo(w1)
    stack = np.concatenate([hi, hi, lo, lo])
    out = np.zeros((4 * d, 2, 128), dtype=np.float32)
    out[:, 0, 0:64] = stack
    out[:, 1, 64:128] = stack
    return out.reshape(4 * d, 256).astype(E4NP)


def _blockdiag2(w):
    """w [d, m] -> [2d, 2m] block-diagonal."""
    d, m = w.shape
    out = np.zeros((2 * d, 2 * m), dtype=np.float32)
    out[:d, :m] = w
    out[d:, m:] = w
    return out


def _build_consts(i):
    np32 = lambda a: np.asarray(a, dtype=np.float32)
    w18 = np.zeros((128, 512), dtype=E4NP)
    l1t = _l1_lhsT(np32(i["tw1"]))
    l1r = _l1_lhsT(np32(i["rw1"]))
    for q in range(4):
        w18[32 * q:32 * q + 28, 0:256] = l1t
        w18[32 * q:32 * q + 24, 256:512] = l1r
    wts = np.zeros((128, WTS_W), dtype=np.float32)
    # L1 lhsT blocks replicated at the 4 row-group offsets
    bd1t = _blockdiag2(np32(i["tw1"]))   # [14, 128]
    bd1r = _blockdiag2(np32(i["rw1"]))   # [12, 128]
    for q in range(4):
        wts[32 * q:32 * q + 14, W1T_C:W1T_C + 128] = bd1t
        wts[32 * q:32 * q + 12, W1R_C:W1R_C + 128] = bd1r
    wts[:, W2T_C:W2T_C + 128] = _blockdiag2(np32(i["tw2"]))
    wts[:, W2R_C:W2R_C + 128] = _blockdiag2(np32(i["rw2"]))
    wts[0:108, MW1_C:MW1_C + 128] = np32(i["mw1"])
    wts[:, MW2_C:MW2_C + 128] = np32(i["mw2"])
    wts[:, MW3_C:MW3_C + 64] = np32(i["mw3"])
    wts[0:64, MW4_C:MW4_C + 1] = np32(i["mw4"])
    wts = wts.astype(MMDT_NP)

    wbf = np.zeros((128, 128), dtype=np.float32)
    wbf[:, 0:64] = _blockdiag2(np32(i["tw3"]))
    wbf[:, 64:128] = _blockdiag2(np32(i["rw3"]))
    wbf = wbf.astype(ml_dtypes.bfloat16)

    bs = np.zeros((128, 8), dtype=np.float32)
    bs[:, BS_TB1] = np.concatenate([np32(i["tb1"]), np32(i["tb1"])])
    bs[:, BS_RB1] = np.concatenate([np32(i["rb1"]), np32(i["rb1"])])
    bs[:, BS_TB2] = np.concatenate([np32(i["tb2"]), np32(i["tb2"])])
    bs[:, BS_RB2] = np.concatenate([np32(i["rb2"]), np32(i["rb2"])])
    # fold pooled e-bias into head L1 bias: c@mw1 picks up b3@mw1 rows
    mb1p = (np32(i["mb1"])
            + np32(i["rb3"]) @ np32(i["mw1"])[44:76]
            + np32(i["tb3"]) @ np32(i["mw1"])[76:108])
    bs[:, BS_MB1] = mb1p
    bs[:, BS_MB2] = np32(i["mb2"])
    bs[0:64, BS_MB3] = np32(i["mb3"])
    bs[0:1, BS_MB4] = np32(i["mb4"])
    return wts, wbf, bs, w18


def kernel(**inputs) -> np.ndarray:
    if "nc" not in _CACHE:
        _CACHE["nc"] = _build_bass()
    nc = _CACHE["nc"]

    wts, wbf, bs, w18 = _build_consts(inputs)
    t0 = np.asarray(inputs["tier0_features"], dtype=np.float32)
    rb = np.asarray(inputs["robot_features"], dtype=np.float32)
    tk = np.asarray(inputs["track_features"], dtype=np.float32)

    in_maps = []
    for c in range(N_CORES):
        s = slice(c * B_C, (c + 1) * B_C)
        in_maps.append({
            "xt": _pack_x(tk[s].reshape(B_C * NT, DT), DT, QT),
            "xr": _pack_x(rb[s].reshape(B_C * NR, DR), DR, QR),
            "t0": np.ascontiguousarray(t0[s].T).astype(MMDT_NP),
            "wts": wts, "wbf": wbf, "bs": bs, "w18": w18,
        })

    res = run_bass_kernel_spmd(nc, in_maps, core_ids=list(range(N_CORES)))
    out = np.concatenate([r["y"][0] for r in res.results])
    return out.astype(np.float32)


if __name__ == "__main__":
    rng = np.random.default_rng(0)
    fake = {
        "tier0_features": rng.standard_normal((B, 44), dtype=np.float32),
        "robot_features": rng.standard_normal((B, NR, DR), dtype=np.float32),
        "track_features": rng.standard_normal((B, NT, DT), dtype=np.float32),
    }
    for n, sh in (("rw1", (6, 64)), ("rw2", (64, 64)), ("rw3", (64, 32)),
                  ("tw1", (7, 64)), ("tw2", (64, 64)), ("tw3", (64, 32)),
                  ("mw1", (108, 128)), ("mw2", (128, 128)),
                  ("mw3", (128, 64)), ("mw4", (64, 1))):
        fake[n] = rng.standard_normal(sh, dtype=np.float32) * 0.2
    for n, sh in (("rb1", 64), ("rb2", 64), ("rb3", 32),
                  ("tb1", 64), ("tb2", 64), ("tb3", 32),
                  ("mb1", 128), ("mb2", 128), ("mb3", 64), ("mb4", 1)):
        fake[n] = rng.standard_normal((sh,), dtype=np.float32) * 0.1
    y = kernel(**fake)
    print("kernel out:", y.shape, y[:4])

